# revision 25
# baseline (speedup 1.0000x reference)
"""Trainium2 Bass kernel v6 for BasicGNNEncoder (gnn_message_passing).

Structure (per core, dst-sharded, SPMD-uniform):
  - fp16 edge/GEMM path (fp32 PSUM accumulation); h state hid-major in
    SBUF: hT [128, npadc] fp16. h_full rank-major, ONE AllGather per
    layer boundary (chunked collectives measured ~35us fixed cost each).
  - The SWDGE gather is the wall (measured: 8.7ns/desc per queue, 4
    queues max, ~105 GB/s ceiling, <=1024 idx per call). So v6
    eliminates padded descriptors (~25% of v2's stream): per (chunk,
    group), edges form floor(min_core/128) exactly-full 128-edge tiles
    per 128-wide dst block, plus per-core DENSE "merged" remainder
    tiles with chunk-wide one-hot. Per-core num_idxs_reg (value_load)
    drops each core's trailing -1 idx slots => no descriptors for pads.
  - Segment-sum via one-hot matmuls into a per-chunk PSUM bank; GRU in
    transposed land; edge GEMM folded into GRU input weights host-side;
    bhh_n via scalar_tensor_tensor, bih_n via tanh activation bias.
  - Queues assigned post-schedule (DMASW sem lanes rotate in final
    program order; queue must equal lane%NQ or sems cross queues).
"""

import hashlib
import os
import sys

import numpy as np

for _p in ("/opt/trn_rl_repo",):
    if _p not in sys.path:
        sys.path.insert(0, _p)

import concourse.bass as bass  # noqa: E402
import concourse.bacc as bacc  # noqa: E402
import concourse.mybir as mybir  # noqa: E402
import concourse.tile as tile  # noqa: E402

P = 128
BLK1 = 128        # full-tile dst block width
CHUNKW = 512
F32 = mybir.dt.float32
F16 = mybir.dt.float16
I16 = mybir.dt.int16
I32 = mybir.dt.int32

NQ = int(os.environ.get("K_NQ", "4"))
MAX_TILES = 8     # HW: num_idxs <= 1024 per dma_gather call
STG_BUFS = int(os.environ.get("K_STG_BUFS", "32"))
OH_BUFS = int(os.environ.get("K_OH_BUFS", "6"))
SEG_BUFS = int(os.environ.get("K_SEG_BUFS", "3"))
GATE_BUFS = int(os.environ.get("K_GATE_BUFS", "3"))
TR_BUFS = int(os.environ.get("K_TR_BUFS", "2"))


def _cdiv(a, b):
    return (a + b - 1) // b


class Schedule:
    pass


class Call:
    pass


# ===========================================================================
# Host-side preprocessing
# ===========================================================================
def _preprocess(edge_index, n_nodes, n_cores):
    src = np.asarray(edge_index[0], dtype=np.int64)
    dst = np.asarray(edge_index[1], dtype=np.int64)

    s = Schedule()
    s.n_nodes = n_nodes
    s.n_cores = n_cores
    s.shard = _cdiv(n_nodes, n_cores)
    s.npadc = _cdiv(s.shard, P) * P
    s.npad_all = s.npadc * n_cores

    # gather groups: contiguous slices of the rank-major global row space
    s.gch = _cdiv(_cdiv(s.npad_all, 4), P) * P
    assert s.gch <= 32767
    s.n_groups = _cdiv(s.npad_all, s.gch)
    s.R_group = [g * s.gch for g in range(s.n_groups)]
    s.group_rows = [
        min(s.gch, s.npad_all - g * s.gch) for g in range(s.n_groups)
    ]

    s.chunks = []
    c0 = 0
    while c0 < s.npadc:
        w = min(CHUNKW, s.npadc - c0)
        s.chunks.append((c0, w))
        c0 += w
    n_chunks = len(s.chunks)

    # ---- per-edge mapping ----
    owner = src // s.shard
    r_loc = src - owner * s.shard
    grow = owner * s.npadc + r_loc
    grp = grow // s.gch
    rig = grow - grp * s.gch             # idx within group (int16-safe)
    core = dst // s.shard
    d = dst - core * s.shard             # local dst row
    ci_e = np.minimum(d // CHUNKW, n_chunks - 1)
    b1 = d // BLK1                       # global 128-block id

    nb1 = s.npadc // BLK1
    # counts per (core, group, block)
    cntb = np.zeros((n_cores, s.n_groups, nb1), np.int64)
    np.add.at(cntb, (core, grp, b1), 1)
    Fb = (cntb.min(axis=0) // P)         # [g, b1] exactly-full tiles

    # remainder sums per (core, group, chunk)
    cntc = np.zeros((n_cores, s.n_groups, n_chunks), np.int64)
    np.add.at(cntc, (core, grp, ci_e), 1)
    rem = np.zeros((n_cores, s.n_groups, n_chunks), np.int64)
    for ci, (c0, w) in enumerate(s.chunks):
        blo, bhi = c0 // BLK1, (c0 + w) // BLK1
        full_e = (Fb[:, blo:bhi] * P).sum(axis=1)      # [g]
        rem[:, :, ci] = cntc[:, :, ci] - full_e[None, :]
    assert (rem >= 0).all()
    Mgc = _cdiv(np.maximum(rem.max(axis=0), 1), P)     # [g, ci] merged tiles

    # ---- build call schedule (chunk-major: for ci: for g) ----
    calls = []          # Call objects
    n_tiles = 0
    n_ft = 0            # full-tile stream counter
    n_mt = 0            # merged-tile stream counter
    n_dyn = 0
    for ci, (c0, w) in enumerate(s.chunks):
        blo, bhi = c0 // BLK1, (c0 + w) // BLK1
        for g in range(s.n_groups):
            rels = []
            for b in range(blo, bhi):
                rels += [b - blo] * int(Fb[g, b])
            M = int(Mgc[g, ci])
            # split into calls of <= MAX_TILES; merged tiles always in
            # the final call (trailing -1 idx slots must be call-trailing)
            nf = len(rels)
            if nf + M <= MAX_TILES:
                parts = [(rels, M)]
            else:
                parts = []
                take = rels
                while len(take) > MAX_TILES:
                    parts.append((take[:MAX_TILES], 0))
                    take = take[MAX_TILES:]
                if len(take) + M <= MAX_TILES:
                    parts.append((take, M))
                else:
                    parts.append((take, 0))
                    parts.append(([], M))
            # merged-carrying part first: the chunk's first matmul must
            # cover the full PSUM bank (zero-region all-or-nothing)
            parts.sort(key=lambda p: 0 if p[1] > 0 else 1)
            for (pr, pm) in parts:
                c = Call()
                c.g, c.ci = g, ci
                c.rels = pr
                c.nf, c.nm = len(pr), pm
                c.bt = c.nf + c.nm
                c.t0 = n_tiles
                c.f0, c.m0 = n_ft, n_mt
                c.dyn = pm > 0
                c.nidx_col = n_dyn if c.dyn else None
                if c.dyn:
                    n_dyn += 1
                n_tiles += c.bt
                n_ft += c.nf
                n_mt += pm
                calls.append(c)
    s.calls = calls
    s.n_tiles, s.n_ft, s.n_mt, s.n_dyn = n_tiles, n_ft, n_mt, n_dyn
    s.max_nm = max((c.nm for c in calls), default=1)

    # ---- per-core slot fill ----
    order = np.lexsort((d, b1, grp, core))
    src_o = rig[order]
    d_o = d[order]
    keyb = ((core * s.n_groups + grp) * nb1 + b1)[order]
    nkb = n_cores * s.n_groups * nb1
    cntk = np.bincount(keyb, minlength=nkb)
    startk = np.concatenate([[0], np.cumsum(cntk)])

    idx_all = np.full((n_cores, n_tiles, P), -1, np.int16)
    dlf = np.full((n_cores, max(n_ft, 1), P), -1.0, np.float16)
    dlm = np.full((n_cores, max(n_mt, 1), P), -1.0, np.float16)
    nidx = np.zeros((n_cores, max(n_dyn, 1)), np.int32)

    # per (core, g, block): how many edges already consumed
    used = np.zeros((n_cores, s.n_groups, nb1), np.int64)

    runs = {}
    for c in calls:
        runs.setdefault((c.ci, c.g), []).append(c)
    for (ci, g), run in runs.items():
        c0, w = s.chunks[ci]
        blo = c0 // BLK1
        for cc in range(n_cores):
            for c in run:
                # fulls: exactly 128 edges each from block rels[j]
                for j, rel in enumerate(c.rels):
                    b = blo + rel
                    k = (cc * s.n_groups + g) * nb1 + b
                    e0 = startk[k] + used[cc, g, b]
                    ee = e0 + P
                    sl = c.t0 + j
                    idx_all[cc, sl, :] = src_o[e0:ee].astype(np.int16)
                    dlf[cc, c.f0 + j, :] = (d_o[e0:ee] - b * BLK1).astype(
                        np.float16
                    )
                    used[cc, g, b] += P
            c = [c for c in run if c.nm]
            assert len(c) <= 1
            c = c[0] if c else None
            if c is not None:
                # merged: dense-pack remaining edges of ALL blocks in chunk
                pool_i = []
                pool_d = []
                for b in range(blo, (c0 + w) // BLK1):
                    k = (cc * s.n_groups + g) * nb1 + b
                    e0 = startk[k] + used[cc, g, b]
                    ee = startk[k + 1]
                    if ee > e0:
                        pool_i.append(src_o[e0:ee])
                        pool_d.append(d_o[e0:ee] - c0)
                        used[cc, g, b] += ee - e0
                if pool_i:
                    pi = np.concatenate(pool_i)
                    pd = np.concatenate(pool_d)
                else:
                    pi = np.zeros(1, np.int64)      # keep >=1 real idx
                    pd = np.full(1, -1.0)
                nreal = len(pi)
                assert nreal <= c.nm * P, (nreal, c.nm)
                for j in range(c.nm):
                    e0, ee = j * P, min((j + 1) * P, nreal)
                    if ee > e0:
                        n = ee - e0
                        sl = c.t0 + c.nf + j
                        idx_all[cc, sl, :n] = pi[e0:ee].astype(np.int16)
                        dlm[cc, c.m0 + j, :n] = pd[e0:ee].astype(np.float16)
                nidx[cc, c.nidx_col] = c.nf * P + nreal
    assert (used.sum(axis=(1, 2)) == np.bincount(core, minlength=n_cores)).all()
    for c in calls:
        if not c.dyn:
            assert (idx_all[:, c.t0 : c.t0 + c.bt] >= 0).all()

    # idx stream in dma_gather layout: [128, 8*n_tiles] int16
    s.idx_arrs = []
    s.dlf_arrs = []
    s.dlm_arrs = []
    for cc in range(n_cores):
        flat = idx_all[cc].reshape(-1)
        cols = flat.reshape(-1, 16).T
        s.idx_arrs.append(np.ascontiguousarray(np.tile(cols, (8, 1))))
        s.dlf_arrs.append(np.ascontiguousarray(dlf[cc].transpose(1, 0)))
        s.dlm_arrs.append(np.ascontiguousarray(dlm[cc].transpose(1, 0)))
    s.nidx = nidx

    # degree per core (padded cols)
    deg = np.zeros((n_cores, s.npadc), np.float16)
    cnt_d = np.bincount(dst, minlength=n_nodes).astype(np.float16)
    for cc in range(n_cores):
        lo = cc * s.shard
        hi = min(n_nodes, (cc + 1) * s.shard)
        deg[cc, : hi - lo] = cnt_d[lo:hi]
    ones = np.ones((n_cores, 1, s.npadc), np.float16)
    s.deg = np.concatenate([deg.reshape(n_cores, 1, s.npadc), ones], axis=1)
    return s


# ===========================================================================
# Program builder
# ===========================================================================
def _build_program(s, feat, hid, n_layers, debug=False):
    assert hid == P and feat % P == 0
    kf = feat // P
    nc = bacc.Bacc(
        "TRN2",
        target_bir_lowering=False,
        debug=debug,
        num_devices=s.n_cores,
        num_swdge_queues=NQ,
    )

    # ---- I/O ----
    xT = nc.dram_tensor("xT", [feat, s.npadc], F16, kind="ExternalInput")
    degt = nc.dram_tensor("deg", [2, s.npadc], F16, kind="ExternalInput")
    dlf_t = nc.dram_tensor(
        "dlf", [P, max(s.n_ft, 1)], F16, kind="ExternalInput"
    )
    dlm_t = nc.dram_tensor(
        "dlm", [P, max(s.n_mt, 1)], F16, kind="ExternalInput"
    )
    idx_t = nc.dram_tensor(
        "idx", [P, 8 * s.n_tiles], I16, kind="ExternalInput"
    )
    nidx_t = nc.dram_tensor(
        "nidx", [1, max(s.n_dyn, 1)], I32, kind="ExternalInput"
    )
    iota1t = nc.dram_tensor(
        "iota1", [P, MAX_TILES * BLK1], F16, kind="ExternalInput"
    )
    iotamt = nc.dram_tensor(
        "iotam", [P, MAX_TILES * CHUNKW], F16, kind="ExternalInput"
    )
    ident = nc.dram_tensor("ident", [P, P], F16, kind="ExternalInput")
    wpT = nc.dram_tensor("wpT", [feat, P], F16, kind="ExternalInput")
    bp = nc.dram_tensor("bp", [P, 1], F32, kind="ExternalInput")
    wfT = nc.dram_tensor("wfT", [n_layers, P, 3 * P], F16, kind="ExternalInput")
    whhT = nc.dram_tensor(
        "whhT", [n_layers, P, 3 * P], F16, kind="ExternalInput"
    )
    bf2 = nc.dram_tensor("bf2", [n_layers, 2, 4 * P], F16, kind="ExternalInput")
    gbact = nc.dram_tensor("gbact", [n_layers, P, 2], F32, kind="ExternalInput")
    gbn = nc.dram_tensor("gbn", [n_layers, P, 2], F32, kind="ExternalInput")
    h_out = nc.dram_tensor("h_out", [s.npadc, P], F32, kind="ExternalOutput")

    h_own = [
        nc.dram_tensor(f"h_own{l}", [s.npadc, P], F16, kind="Internal")
        for l in range(n_layers)
    ]
    h_full = [
        nc.dram_tensor(
            f"h_full{l}", [s.npad_all, P], F16, addr_space="Shared"
        )
        for l in range(n_layers)
    ]
    rg = [list(range(s.n_cores))]

    from contextlib import ExitStack

    with tile.TileContext(nc) as tc, ExitStack() as ctx:
        consts = ctx.enter_context(tc.tile_pool(name="consts", bufs=1))
        sb_in = ctx.enter_context(tc.tile_pool(name="sb_in", bufs=3))
        sb_stg = ctx.enter_context(tc.tile_pool(name="sb_stg", bufs=2))
        sb_gru = ctx.enter_context(tc.tile_pool(name="sb_gru", bufs=2))
        sb_st = ctx.enter_context(tc.tile_pool(name="sb_st", bufs=3))
        psum = ctx.enter_context(
            tc.tile_pool(name="psum", bufs=2, space="PSUM")
        )

        # ---- constants ----
        iota1_sb = consts.tile(
            [P, MAX_TILES * BLK1], F16, tag="iota1", name="iota1_sb"
        )
        nc.sync.dma_start(out=iota1_sb[:], in_=iota1t[:, :])
        iotam_sb = consts.tile(
            [P, MAX_TILES * CHUNKW], F16, tag="iotam", name="iotam_sb"
        )
        nc.sync.dma_start(out=iotam_sb[:], in_=iotamt[:, :])
        iden_sb = consts.tile([P, P], F16, tag="iden", name="iden_sb")
        nc.sync.dma_start(out=iden_sb[:], in_=ident[:, :])
        wp_sb = [
            consts.tile([P, P], F16, tag=f"wp{k}", name=f"wp_sb{k}")
            for k in range(kf)
        ]
        for k in range(kf):
            nc.sync.dma_start(out=wp_sb[k][:], in_=wpT[k * P : (k + 1) * P, :])
        bp_sb = consts.tile([P, 1], F32, tag="bp", name="bp_sb")
        nc.sync.dma_start(out=bp_sb[:], in_=bp[:, :])
        wf_sb = [
            consts.tile([P, 3 * P], F16, tag=f"wf{l}", name=f"wf_sb{l}")
            for l in range(n_layers)
        ]
        whh_sb = [
            consts.tile([P, 3 * P], F16, tag=f"whh{l}", name=f"whh_sb{l}")
            for l in range(n_layers)
        ]
        bf2_sb = [
            consts.tile([2, 4 * P], F16, tag=f"bf2{l}", name=f"bf2_sb{l}")
            for l in range(n_layers)
        ]
        gba_sb = [
            consts.tile([P, 2], F32, tag=f"gba{l}", name=f"gba_sb{l}")
            for l in range(n_layers)
        ]
        gbn_sb = [
            consts.tile([P, 2], F32, tag=f"gbn{l}", name=f"gbn_sb{l}")
            for l in range(n_layers)
        ]
        for l in range(n_layers):
            nc.sync.dma_start(out=wf_sb[l][:], in_=wfT[l])
            nc.sync.dma_start(out=whh_sb[l][:], in_=whhT[l])
            nc.sync.dma_start(out=bf2_sb[l][:], in_=bf2[l])
            nc.sync.dma_start(out=gba_sb[l][:], in_=gbact[l])
            nc.sync.dma_start(out=gbn_sb[l][:], in_=gbn[l])
        dgo = consts.tile([2, s.npadc], F16, tag="dgo", name="dgo")
        nc.sync.dma_start(out=dgo[:, :], in_=degt[:, :])
        idx_sb = consts.tile([P, 8 * s.n_tiles], I16, tag="idx", name="idx_sb")
        nc.sync.dma_start(out=idx_sb[:], in_=idx_t[:, :])
        dlf_sb = consts.tile(
            [P, max(s.n_ft, 1)], F16, tag="dlf", name="dlf_sb"
        )
        nc.sync.dma_start(out=dlf_sb[:], in_=dlf_t[:, :])
        dlm_sb = consts.tile(
            [P, max(s.n_mt, 1)], F16, tag="dlm", name="dlm_sb"
        )
        nc.sync.dma_start(out=dlm_sb[:], in_=dlm_t[:, :])
        nidx_sb = consts.tile(
            [1, max(s.n_dyn, 1)], I32, tag="nidx", name="nidx_sb"
        )
        nc.sync.dma_start(out=nidx_sb[:], in_=nidx_t[:, :])

        sb_big = ctx.enter_context(tc.tile_pool(name="sb_big", bufs=1))
        hT = sb_big.tile([P, s.npadc], F16, tag="hT", name="hT")
        nidx_reg = nc.gpsimd.alloc_register("nidx_reg")

        def transpose_store(dst_dram, c0, w, cast_dt, on_vector=False):
            tp = psum.tile([P, CHUNKW], F16, tag="tr", name="tp", bufs=TR_BUFS)
            for j in range(w // P):
                nc.tensor.transpose(
                    out=tp[:, j * P : (j + 1) * P],
                    in_=hT[:, c0 + j * P : c0 + (j + 1) * P],
                    identity=iden_sb[:],
                )
            st = sb_st.tile([P, CHUNKW], cast_dt, tag="tst", name="tst")
            if on_vector:
                nc.vector.tensor_copy(out=st[:, :w], in_=tp[:, :w])
            else:
                nc.scalar.copy(out=st[:, :w], in_=tp[:, :w])
            out_ap = dst_dram[c0 : c0 + w, :].rearrange(
                "(j a) b -> a j b", a=P
            )
            in_ap = st[:, :w].rearrange("a (j b) -> a j b", b=P)
            nc.sync.dma_start(out=out_ap, in_=in_ap)

        def allgather(l):
            nc.gpsimd.collective_compute(
                "AllGather",
                mybir.AluOpType.bypass,
                replica_groups=rg,
                ins=[h_own[l][:, :]],
                outs=[h_full[l][:, :]],
            )

        # ---- projection ----
        for ci, (c0, w) in enumerate(s.chunks):
            xa = sb_in.tile([P, kf, CHUNKW], F16, tag="xa", name="xa")
            for k in range(kf):
                nc.sync.dma_start(
                    out=xa[:, k, :w], in_=xT[k * P : (k + 1) * P, c0 : c0 + w]
                )
            ps = psum.tile(
                [P, CHUNKW], F32, tag="seg", name="ps_seg", bufs=SEG_BUFS
            )
            for k in range(kf):
                nc.tensor.matmul(
                    out=ps[:, :w],
                    lhsT=wp_sb[k][:],
                    rhs=xa[:, k, :w],
                    start=(k == 0),
                    stop=(k == kf - 1),
                )
            nc.scalar.activation(
                out=hT[:, c0 : c0 + w],
                in_=ps[:, :w],
                func=mybir.ActivationFunctionType.Relu,
                bias=bp_sb[:, 0:1],
            )
            transpose_store(h_own[0], c0, w, F16, on_vector=True)
        allgather(0)

        # ---- memset stg rings once (skipped gather slots leave stale
        # data; must be finite so one-hot zeros kill it) ----
        for _bi in range(STG_BUFS):
            t = sb_stg.tile(
                [P, MAX_TILES, P], F16, tag="stg", name="stg",
                bufs=STG_BUFS,
            )
            nc.vector.memset(t[:], 0.0)

        MAXM = max(s.max_nm, 1)
        # group calls by chunk
        calls_of_chunk = {}
        for c in s.calls:
            calls_of_chunk.setdefault(c.ci, []).append(c)

        # ---- layers ----
        for l in range(n_layers):
            hf = h_full[l]
            for ci, (c0, w) in enumerate(s.chunks):
                sl_c = slice(c0, c0 + w)
                ccalls = calls_of_chunk[ci]
                n_mm = sum(c.bt for c in ccalls)
                mm = 0
                ps_seg = psum.tile(
                    [P, CHUNKW], F32, tag="seg", name="ps_seg", bufs=SEG_BUFS
                )
                for c in ccalls:
                    stg = sb_stg.tile(
                        [P, MAX_TILES, P], F16, tag="stg", name="stg",
                        bufs=STG_BUFS,
                    )
                    if c.dyn:
                        nc.gpsimd.reg_load(
                            nidx_reg,
                            nidx_sb[0:1, c.nidx_col : c.nidx_col + 1],
                        )
                        nreg = nidx_reg
                    else:
                        nreg = P * c.bt
                    nc.gpsimd.dma_gather(
                        stg[:, : c.bt, :],
                        hf[s.R_group[c.g] : s.R_group[c.g]
                           + s.group_rows[c.g], :],
                        idx_sb[:, 8 * c.t0 : 8 * (c.t0 + c.bt)],
                        num_idxs=P * c.bt,
                        num_idxs_reg=nreg,
                        elem_size=P,
                        queue_num=0,  # provisional; reassigned post-schedule
                    )
                    ohf = None
                    ohm = None
                    if c.nf:
                        ohf = sb_in.tile(
                            [P, MAX_TILES * BLK1], F16, tag="ohf",
                            name="ohf", bufs=OH_BUFS,
                        )
                        nc.vector.tensor_tensor(
                            out=ohf[:, : c.nf * BLK1].rearrange(
                                "p (t j) -> p t j", j=BLK1
                            ),
                            in0=dlf_sb[
                                :, c.f0 : c.f0 + c.nf, None
                            ].to_broadcast([P, c.nf, BLK1]),
                            in1=iota1_sb[:, : c.nf * BLK1].rearrange(
                                "p (t j) -> p t j", j=BLK1
                            ),
                            op=mybir.AluOpType.is_equal,
                        )
                    if c.nm:
                        ohm = sb_in.tile(
                            [P, MAXM * CHUNKW], F16, tag="ohm",
                            name="ohm", bufs=OH_BUFS,
                        )
                        nc.vector.tensor_tensor(
                            out=ohm[:, : c.nm * CHUNKW].rearrange(
                                "p (t j) -> p t j", j=CHUNKW
                            ),
                            in0=dlm_sb[
                                :, c.m0 : c.m0 + c.nm, None
                            ].to_broadcast([P, c.nm, CHUNKW]),
                            in1=iotam_sb[:, : c.nm * CHUNKW].rearrange(
                                "p (t j) -> p t j", j=CHUNKW
                            ),
                            op=mybir.AluOpType.is_equal,
                        )
                    jorder = list(range(c.nf, c.bt)) + list(range(c.nf))
                    for j in jorder:
                        if j < c.nf:
                            rel = c.rels[j]
                            out_ap = ps_seg[
                                :, rel * BLK1 : (rel + 1) * BLK1
                            ]
                            rhs = ohf[:, j * BLK1 : (j + 1) * BLK1]
                        else:
                            out_ap = ps_seg[:, :CHUNKW]
                            jm = j - c.nf
                            rhs = ohm[
                                :, jm * CHUNKW : (jm + 1) * CHUNKW
                            ]
                        nc.tensor.matmul(
                            out=out_ap,
                            lhsT=stg[:, j, :],
                            rhs=rhs,
                            start=(mm == 0),
                            stop=(mm == n_mm - 1),
                            skip_group_check=True,
                        )
                        mm += 1

                # ---- GRU update for chunk ci ----
                aggc = sb_gru.tile(
                    [P, CHUNKW], F16, tag="aggc", name="aggc", bufs=3
                )
                nc.vector.tensor_copy(out=aggc[:, :w], in_=ps_seg[:, :w])

                def gate_ps(col, with_agg, with_h, with_bias=True):
                    pg = psum.tile(
                        [P, CHUNKW], F32, tag="gate", name="pg",
                        bufs=GATE_BUFS,
                    )
                    if with_agg:
                        nc.tensor.matmul(
                            out=pg[:, :w],
                            lhsT=wf_sb[l][:, col * P : (col + 1) * P],
                            rhs=aggc[:, :w],
                            start=True,
                            stop=False,
                        )
                    if with_h:
                        hcol = 2 * P if col == 3 else col * P
                        nc.tensor.matmul(
                            out=pg[:, :w],
                            lhsT=whh_sb[l][:, hcol : hcol + P],
                            rhs=hT[:, sl_c],
                            start=not with_agg,
                            stop=not with_bias,
                        )
                    if with_bias:
                        nc.tensor.matmul(
                            out=pg[:, :w],
                            lhsT=bf2_sb[l][:, col * P : (col + 1) * P],
                            rhs=dgo[:, sl_c],
                            start=False,
                            stop=True,
                        )
                    return pg

                pr = gate_ps(0, True, True)
                r = sb_gru.tile([P, CHUNKW], F16, tag="r", name="rt")
                nc.scalar.activation(
                    out=r[:, :w], in_=pr[:, :w],
                    func=mybir.ActivationFunctionType.Sigmoid,
                    bias=gba_sb[l][:, 0:1],
                )
                pz = gate_ps(1, True, True)
                z = sb_gru.tile([P, CHUNKW], F16, tag="z", name="zt")
                nc.scalar.activation(
                    out=z[:, :w], in_=pz[:, :w],
                    func=mybir.ActivationFunctionType.Sigmoid,
                    bias=gba_sb[l][:, 1:2],
                )
                pi = gate_ps(2, True, False)
                ph = gate_ps(3, False, True, with_bias=False)
                # t1 = r * (ph + bhh_n) + pi ; n = tanh(t1 + bih_n)
                t1 = sb_gru.tile([P, CHUNKW], F32, tag="t1", name="t1")
                nc.vector.scalar_tensor_tensor(
                    out=t1[:, :w], in0=ph[:, :w],
                    scalar=gbn_sb[l][:, 1:2], in1=r[:, :w],
                    op0=mybir.AluOpType.add,
                    op1=mybir.AluOpType.mult,
                )
                nc.vector.tensor_add(out=t1[:, :w], in0=t1[:, :w], in1=pi[:, :w])
                n_t = sb_gru.tile([P, CHUNKW], F16, tag="nt", name="n_t")
                nc.scalar.activation(
                    out=n_t[:, :w], in_=t1[:, :w],
                    func=mybir.ActivationFunctionType.Tanh,
                    bias=gbn_sb[l][:, 0:1],
                )
                t3 = sb_gru.tile([P, CHUNKW], F16, tag="t3", name="t3")
                nc.vector.tensor_sub(out=t3[:, :w], in0=hT[:, sl_c], in1=n_t[:, :w])
                nc.vector.tensor_mul(out=t3[:, :w], in0=z[:, :w], in1=t3[:, :w])
                nc.vector.tensor_add(out=hT[:, sl_c], in0=n_t[:, :w], in1=t3[:, :w])

                if l < n_layers - 1:
                    transpose_store(h_own[l + 1], c0, w, F16)
                else:
                    transpose_store(h_out, c0, w, F32)

            if l < n_layers - 1:
                allgather(l + 1)

    # Post-schedule queue assignment: DMASW sem lanes rotate in FINAL
    # program order; queue must equal lane%NQ or a sem is touched from
    # two queues (HW deadlock).
    from concourse.tile_scheduler import DMAInst as _DMAInst
    lane = 0
    for _blk in nc.m.functions[0].blocks:
        for _inst in _blk.instructions:
            if _inst.engine == mybir.EngineType.Pool and isinstance(
                _inst, _DMAInst
            ):
                _inst.queue_num = lane % NQ
                lane += 1

    nc.compile()
    return nc


# ===========================================================================
# Input packing
# ===========================================================================
def _make_in_maps(s, inputs, feat, hid, n_layers):
    nf = np.asarray(inputs["node_features"], np.float32)
    w_proj = np.asarray(inputs["w_proj"], np.float64)
    b_proj = np.asarray(inputs["b_proj"], np.float64)
    edge_w = np.asarray(inputs["edge_w"], np.float64)
    edge_b = np.asarray(inputs["edge_b"], np.float64)
    gru_wih = np.asarray(inputs["gru_wih"], np.float64)
    gru_whh = np.asarray(inputs["gru_whh"], np.float64)
    gru_bih = np.asarray(inputs["gru_bih"], np.float64)
    gru_bhh = np.asarray(inputs["gru_bhh"], np.float64)

    n_nodes = nf.shape[0]
    xT = np.zeros((feat, s.npad_all), np.float16)
    xTv = nf.T
    for c in range(s.n_cores):
        lo = c * s.shard
        hi = min(n_nodes, (c + 1) * s.shard)
        xT[:, c * s.npadc : c * s.npadc + hi - lo] = xTv[:, lo:hi]

    iota1 = np.tile(
        np.arange(BLK1, dtype=np.float16), MAX_TILES
    )[None, :].repeat(P, 0)
    iotam = np.tile(
        np.arange(CHUNKW, dtype=np.float16), MAX_TILES
    )[None, :].repeat(P, 0)
    ident = np.eye(P, dtype=np.float16)
    wpT = np.ascontiguousarray(w_proj.T).astype(np.float16)
    bp = b_proj.reshape(P, 1).astype(np.float32)

    ew = edge_w[:, 0]
    eb = edge_b[:, 0]
    wfT = np.zeros((n_layers, P, 3 * P), np.float16)
    whhT = np.zeros((n_layers, P, 3 * P), np.float16)
    bf2 = np.zeros((n_layers, 2, 4 * P), np.float16)
    gba = np.zeros((n_layers, P, 2), np.float32)
    gbn = np.zeros((n_layers, P, 2), np.float32)
    for l in range(n_layers):
        wf = gru_wih[l] @ ew[l]
        bf = gru_wih[l] @ eb[l]
        wfT[l] = wf.T.astype(np.float16)
        whhT[l] = gru_whh[l].T.astype(np.float16)
        bf2[l, 0, 0:P] = bf[0:P]
        bf2[l, 0, P : 2 * P] = bf[P : 2 * P]
        bf2[l, 0, 2 * P : 3 * P] = bf[2 * P : 3 * P]
        gba[l, :, 0] = gru_bih[l, 0:P] + gru_bhh[l, 0:P]
        gba[l, :, 1] = gru_bih[l, P : 2 * P] + gru_bhh[l, P : 2 * P]
        gbn[l, :, 0] = gru_bih[l, 2 * P : 3 * P]
        gbn[l, :, 1] = gru_bhh[l, 2 * P : 3 * P]

    in_maps = []
    for c in range(s.n_cores):
        m = {
            "xT": np.ascontiguousarray(xT[:, c * s.npadc : (c + 1) * s.npadc]),
            "deg": s.deg[c],
            "dlf": s.dlf_arrs[c],
            "dlm": s.dlm_arrs[c],
            "idx": s.idx_arrs[c],
            "nidx": s.nidx[c : c + 1],
            "iota1": iota1,
            "iotam": iotam,
            "ident": ident,
            "wpT": wpT,
            "bp": bp,
            "wfT": wfT,
            "whhT": whhT,
            "bf2": bf2,
            "gbact": gba,
            "gbn": gbn,
        }
        in_maps.append(m)
    return in_maps


# ===========================================================================
# Public entry point
# ===========================================================================
_CACHE = {}


def _get_compiled(edge_index, n_nodes, feat, hid, n_layers, n_cores=8):
    key = hashlib.sha1(
        np.ascontiguousarray(edge_index).tobytes()
        + np.int64([n_nodes, feat, hid, n_layers, n_cores, 6,
                    NQ, STG_BUFS, OH_BUFS, SEG_BUFS, GATE_BUFS, TR_BUFS]
                   ).tobytes()
    ).hexdigest()
    if key not in _CACHE:
        s = _preprocess(edge_index, n_nodes, n_cores)
        nc = _build_program(s, feat, hid, n_layers, debug=False)
        _CACHE[key] = (s, nc)
    return _CACHE[key]


def run(inputs, trace=False, tmpdir=None):
    from concourse.bass_utils import run_bass_kernel_spmd

    nf = np.asarray(inputs["node_features"])
    edge_index = np.asarray(inputs["edge_index"])
    n_nodes, feat = nf.shape
    hid = np.asarray(inputs["w_proj"]).shape[0]
    n_layers = np.asarray(inputs["gru_wih"]).shape[0]
    s, nc = _get_compiled(edge_index, n_nodes, feat, hid, n_layers)
    in_maps = _make_in_maps(s, inputs, feat, hid, n_layers)
    res = run_bass_kernel_spmd(
        nc, in_maps, core_ids=list(range(s.n_cores)), trace=trace,
        tmpdir=tmpdir,
    )
    out = np.empty((n_nodes, hid), np.float32)
    for c in range(s.n_cores):
        lo = c * s.shard
        hi = min(n_nodes, (c + 1) * s.shard)
        out[lo:hi] = res.results[c]["h_out"][: hi - lo]
    return out, res


def kernel(**inputs) -> np.ndarray:
    out, _ = run(inputs, trace=False)
    return out


# ===========================================================================
# Small-scale CoreSim self-test
# ===========================================================================
def _np_reference(inputs, n_layers):
    nf = np.asarray(inputs["node_features"], np.float64)
    src, dst = np.asarray(inputs["edge_index"], np.int64)
    w_proj = np.asarray(inputs["w_proj"], np.float64)
    h = np.maximum(nf @ w_proj.T + np.asarray(inputs["b_proj"], np.float64), 0)
    n = nf.shape[0]

    def sig(x):
        return 1.0 / (1.0 + np.exp(-x))

    for l in range(n_layers):
        ew = np.asarray(inputs["edge_w"], np.float64)[l, 0]
        ebv = np.asarray(inputs["edge_b"], np.float64)[l, 0]
        agg = np.zeros_like(h)
        np.add.at(agg, dst, h[src])
        deg = np.bincount(dst, minlength=n).astype(np.float64)[:, None]
        agg = agg @ ew.T + deg * ebv
        wih = np.asarray(inputs["gru_wih"], np.float64)[l]
        whh = np.asarray(inputs["gru_whh"], np.float64)[l]
        bih = np.asarray(inputs["gru_bih"], np.float64)[l]
        bhh = np.asarray(inputs["gru_bhh"], np.float64)[l]
        gi = agg @ wih.T + bih
        gh = h @ whh.T + bhh
        H = h.shape[1]
        r = sig(gi[:, :H] + gh[:, :H])
        z = sig(gi[:, H : 2 * H] + gh[:, H : 2 * H])
        nn_ = np.tanh(gi[:, 2 * H :] + r * gh[:, 2 * H :])
        h = (1 - z) * nn_ + z * h
    return h


def _selftest(n_nodes=3000, n_edges=20000, feat=256, hid=128, n_layers=2):
    import os
    from concourse.bass_interp import MultiCoreSim

    rng = np.random.default_rng(0)
    sc = 0.05
    inputs = {
        "node_features": rng.standard_normal((n_nodes, feat)).astype(np.float32),
        "edge_index": rng.integers(0, n_nodes, (2, n_edges), dtype=np.int64).astype(np.int32),
        "edge_type": np.zeros(n_edges, np.int32),
        "w_proj": (rng.standard_normal((hid, feat)) * sc).astype(np.float32),
        "b_proj": (rng.standard_normal(hid) * sc).astype(np.float32),
        "edge_w": (rng.standard_normal((n_layers, 1, hid, hid)) * sc).astype(np.float32),
        "edge_b": (rng.standard_normal((n_layers, 1, hid)) * sc).astype(np.float32),
        "gru_wih": (rng.standard_normal((n_layers, 3 * hid, hid)) * sc).astype(np.float32),
        "gru_whh": (rng.standard_normal((n_layers, 3 * hid, hid)) * sc).astype(np.float32),
        "gru_bih": (rng.standard_normal((n_layers, 3 * hid)) * sc).astype(np.float32),
        "gru_bhh": (rng.standard_normal((n_layers, 3 * hid)) * sc).astype(np.float32),
    }
    edge_index = inputs["edge_index"]
    s = _preprocess(edge_index, n_nodes, 8)
    print(
        f"schedule: tiles={s.n_tiles} (full={s.n_ft} merged={s.n_mt}) "
        f"calls={len(s.calls)} dyn={s.n_dyn} npadc={s.npadc} "
        f"groups={s.n_groups}"
    )
    nc = _build_program(s, feat, hid, n_layers, debug=False)
    in_maps = _make_in_maps(s, inputs, feat, hid, n_layers)

    exp = _np_reference(inputs, n_layers)
    out = np.empty((n_nodes, hid), np.float32)
    if os.environ.get("SELFTEST_HW", "0") == "1":
        from concourse.bass_utils import run_bass_kernel_spmd

        res = run_bass_kernel_spmd(nc, in_maps, core_ids=list(range(8)))
        for c in range(8):
            lo = c * s.shard
            hi = min(n_nodes, (c + 1) * s.shard)
            out[lo:hi] = res.results[c]["h_out"][: hi - lo]
    else:
        sim = MultiCoreSim(nc, 8)
        stg_names = [
            getattr(a, "name", "").removesuffix("_set")
            for a in nc.m.functions[0].allocations
            if getattr(a, "name", "").startswith("stg")
        ]
        for c in range(8):
            for k, v in in_maps[c].items():
                sim.cores[c].tensor(k)[:] = v
            # CoreSim NaN-poisons fresh tile names; on HW the ring memset
            # keeps physical slots finite. Mirror that here.
            for t in stg_names:
                try:
                    sim.cores[c].tensor(t)[:] = 0
                except Exception:
                    pass
        sim.simulate()
        for c in range(8):
            lo = c * s.shard
            hi = min(n_nodes, (c + 1) * s.shard)
            out[lo:hi] = sim.cores[c].mem_tensor("h_out")[: hi - lo]
    err = np.abs(out - exp).max() / max(1e-12, np.abs(exp).max())
    print("selftest rel absmax err:", err)
    assert err < 3e-3, err
    print("SELFTEST PASSED")


if __name__ == "__main__":
    _selftest()
